# revision 31
# baseline (speedup 1.0000x reference)
"""Causal self-attention TRN2 Bass kernel (v2).

Problem: B=2, T=4096, D_MODEL=512, N_HEADS=8, HEAD_DIM=64 (fp32).

Sharding (tensor+data parallel): 8 cores = 2 batches x 4 head-pairs.
Core c handles batch b = c//4 and heads (2g, 2g+1) with g = c%4, over the
full sequence. Each core computes a full-shape [T, 512] partial output
(its two heads' contribution through W_O); the host sums 4 partials per
batch.

v2 structure (single fused loop over query-blocks J of 512 queries; the
whole program is one software pipeline so the PE never idles long enough
for the HAM clock-gate to re-throttle it to 1.2 GHz):
  - the QKV projection for block J+1 is emitted piecewise INSIDE block
    J's chunk loop (iterations K=1..4) on a dedicated 1-bank PSUM tag,
    with the x-tile DMA split across the sync and gpsimd queues so input
    prefetch never queues behind epilogue DMAs.
  - attention: per 128-key chunk, row-tiled score pair (head0 rows 0:64,
    head1 rows 64:128, concurrent), then exp, alternating per chunk
    between ScalarE (exact: exp(s/8 - 4), with a multiplicative GpSimd
    causal-mask on diagonal blocks) and DVE (Schraudolph bit-trick: one
    scalar_tensor_tensor computes uint16(round(s*23.083 + 15511.7)),
    reinterpreted as bf16 ~= exp(s/8-4) within ~3%; for diagonal chunks
    the bias operand is a triangle tile whose masked entries hold 4000,
    making exp ~2^-96, i.e. exact-zero contribution). The global -4
    offset cancels in softmax normalization and keeps exp(max score)
    finite in bf16. Splitting exp across two engines removes the
    ~150us ScalarE wall a single-engine softmax would hit.
  - PV matmuls M=65 (64 dims + ones column accumulating denominators
    for free), software-pipelined 3 chunks behind the score matmuls.
  - epilogue per block pipelined into the NEXT block's chunk loop:
    aoU copies right after the last PV (frees single-buffered PSUM
    accumulators), denominator-row broadcast via K=1 matmuls, fp32
    reciprocals, bf16 normalize muls, head1 partition-shift via
    gpsimd-issued SBUF DMA, W_O projection, output staged through SBUF
    alternating ScalarE/DVE with engine-issued output DMAs.
"""

import math

import ml_dtypes
import numpy as np

import concourse.bass as bass
import concourse.mybir as mybir
import concourse.tile as tile
from concourse.tile import add_dep_helper
from concourse import bacc
from concourse.bass import ds, ts
from concourse.bass_utils import run_bass_kernel_spmd

FP32 = mybir.dt.float32
BF16 = mybir.dt.bfloat16
U16 = mybir.dt.uint16
FP16 = mybir.dt.float16
AF = mybir.ActivationFunctionType
ALU = mybir.AluOpType

T = 4096
DM = 512
QC = 512  # query-block width
KC = 128  # key-chunk width (partition dim)

# Schraudolph bf16-bit exp: bits = s_raw * SMUL + SBIAS, as uint16 -> bf16.
# value ~= exp(s_raw/8 - 4) (matches the ScalarE path's scale/bias).
LOG2E = 1.4426950408889634
SMUL = 128.0 * LOG2E / 8.0  # 23.0831...
SBIAS = 16256.0 - 5.61 - 4.0 * 128.0 * LOG2E  # 15511.73
MASKVAL = 4000.0  # masked-entry bias: exp -> ~2^-96, exactly negligible

TRACE = False
LAST_RESULTS = None


def build_program(t=T):
    assert t % QC == 0
    nb = t // QC
    nkc = t // KC
    nc = bacc.Bacc("TRN2", target_bir_lowering=False, debug=False)

    xT = nc.dram_tensor("xT", [DM, t], BF16, kind="ExternalInput").ap()
    wq = nc.dram_tensor("wq", [DM, 128], BF16, kind="ExternalInput").ap()
    wk = nc.dram_tensor("wk", [DM, 128], BF16, kind="ExternalInput").ap()
    wv = nc.dram_tensor("wv", [DM, 128], BF16, kind="ExternalInput").ap()
    woT = nc.dram_tensor("woT", [128, DM], BF16, kind="ExternalInput").ap()
    outp = nc.dram_tensor("outp", [t, DM], FP16, kind="ExternalOutput").ap()

    with tile.TileContext(nc) as tc:
        with (
            tc.tile_pool(name="consts", bufs=1) as cpool,
            tc.tile_pool(name="persist", bufs=1) as ppool,
            tc.tile_pool(name="xtl", bufs=3) as xpool,
            tc.tile_pool(name="work", bufs=2) as wpool,
            tc.tile_pool(name="ps_sc", bufs=2, space="PSUM") as ps_sc,
            tc.tile_pool(name="ps_pv", bufs=1, space="PSUM") as ps_pv,
            tc.tile_pool(name="ps_ep", bufs=1, space="PSUM") as ps_ep,
            tc.tile_pool(name="ps_pj", bufs=1, space="PSUM") as ps_pj,
        ):
            # ---- constants ----
            wq_s = cpool.tile([128, 512], BF16, name="wq_s")
            wk_s = cpool.tile([128, 512], BF16, name="wk_s")
            wv_s = cpool.tile([128, 512], BF16, name="wv_s")
            woT_s = cpool.tile([128, 512], BF16, name="woT_s")
            nc.sync.dma_start(
                wq_s[:].rearrange("p (d c) -> p d c", d=4),
                wq.rearrange("(d p) c -> p d c", p=128),
            )
            nc.gpsimd.dma_start(
                wk_s[:].rearrange("p (d c) -> p d c", d=4),
                wk.rearrange("(d p) c -> p d c", p=128),
            )

            # Schraudolph bias tiles: plain (off-diagonal) via memset,
            # diagonal-masked variants host-precomputed and DMA'd
            bias_p = cpool.tile([128, 1024], FP32, name="bias_p")
            nc.vector.memset(bias_p[:], SBIAS)
            # diagonal bias tile, slice-relative: triangle (MASKVAL where
            # k > c) in cols [0,128), SBIAS elsewhere; same for both heads
            bias_t = cpool.tile([128, 1024], FP32, name="bias_t")
            nc.vector.memset(bias_t[:], SBIAS)

            # multiplicative causal mask for diagonal blocks of P^T [k, q]:
            # 1 where k <= q, 0 elsewhere (applied to exp output on GpSimd)
            mask_s = cpool.tile([128, 128], BF16, name="mask_s")

            # ones row at partition 64 for the K=1 denominator broadcasts
            ones_bc = cpool.tile([65, 64], BF16, name="ones_bc")
            nc.vector.memset(ones_bc[:], 1.0)

            # per-partition bias vector for the ScalarE exp path (-4 offset)
            negoff = cpool.tile([128, 1], FP32, name="negoff")
            nc.vector.memset(negoff[:], -4.0)

            # ---- persistent activations ----
            qT_s = ppool.tile([128, t], BF16, name="qT_s")
            kT_s = ppool.tile([128, t], BF16, name="kT_s")
            # V natural per chunk/head: [key-in-chunk, chunk, head, 65]
            # (64 dims + ones column; memset 1.0 once)
            v_s = ppool.tile([128, nkc, 2, 65], BF16, name="v_s")
            nc.vector.memset(v_s[:], 1.0)

            inv_sqrt_d = 1.0 / math.sqrt(64.0)

            def emit_proj_dma(J):
                xt = xpool.tile([128, 4, 512], BF16, tag="xt", name="xt")
                src = xT.rearrange("(d p) t -> p d t", p=128)[:, :, ts(J, 512)]
                nc.sync.dma_start(xt[:, 0:2, :], src[:, 0:2, :])
                nc.gpsimd.dma_start(xt[:, 2:4, :], src[:, 2:4, :])
                return xt

            def emit_proj_piece(J, piece, xt, pool=None, tag="pj"):
                # piece 0: Q-proj, 1: K-proj, 2: V tokens 0:256, 3: V 256:512
                pool = pool or ps_pj
                if piece in (0, 1):
                    w_s = (wq_s, wk_s)[piece]
                    psq = pool.tile([128, 512], FP32, tag=tag, name="psq")
                    for d in range(4):
                        nc.tensor.matmul(
                            psq[:],
                            lhsT=w_s[:, ts(d, 128)],
                            rhs=xt[:, d, :],
                            start=(d == 0),
                            stop=(d == 3),
                        )
                    dst = (qT_s, kT_s)[piece]
                    nc.scalar.copy(dst[:, ts(J, 512)], psq[:])
                else:
                    psv = pool.tile([128, 512], FP32, tag=tag, name="psv")
                    for tt2 in range(2):
                        tt = (piece - 2) * 2 + tt2
                        for d in range(4):
                            nc.tensor.matmul(
                                psv[:, ds(tt2 * 256, 128)],
                                lhsT=xt[:, d, ts(tt, 128)],
                                rhs=wv_s[:, ts(d, 128)],
                                start=(d == 0),
                                stop=(d == 3),
                            )
                    for tt2 in range(2):
                        kk = 4 * J + (piece - 2) * 2 + tt2
                        nc.vector.tensor_copy(
                            v_s[:, kk, :, 0:64],
                            psv[:, ds(tt2 * 256, 128)].rearrange(
                                "p (h d) -> p h d", h=2
                            ),
                        )

            # epilogue state: (J, po0, po1) -> after aoU copies: (J, aoU0, aoU1)
            def emit_aou(J, po0, po1):
                aoU0 = wpool.tile([65, 512], BF16, tag="ao0", name="aoU0")
                aoU1 = wpool.tile([65, 512], BF16, tag="ao1", name="aoU1")
                nc.scalar.copy(aoU0[:], po0[:])
                nc.vector.tensor_copy(aoU1[:], po1[:])
                return aoU0, aoU1

            def emit_bcast(J, aoU0, aoU1, last=False):
                # broadcast each denominator row to 64 partitions (K=1
                # matmuls), reciprocal, then normalize (all base-partition 0)
                psb0 = ps_ep.tile([64, 512], FP32, tag="ep", name="psb0")
                nc.tensor.matmul(
                    psb0[:],
                    lhsT=ones_bc[64:65, :],
                    rhs=aoU0[64:65, :],
                    start=True,
                    stop=True,
                )
                if last:
                    psb1 = ps_pj.tile([64, 512], FP32, tag="pj", name="psb1")
                else:
                    psb1 = ps_ep.tile([64, 512], FP32, tag="ep", name="psb1")
                nc.tensor.matmul(
                    psb1[:],
                    lhsT=ones_bc[64:65, :],
                    rhs=aoU1[64:65, :],
                    start=True,
                    stop=True,
                )
                rbc1 = wpool.tile([64, 512], FP32, tag="rbc1", name="rbc1")
                nc.vector.reciprocal_approx_fast(rbc1[:], psb1[:])
                aoT1 = wpool.tile([64, 512], BF16, tag="aoT1", name="aoT1")
                nc.vector.tensor_mul(aoT1[:], aoU1[0:64, :], rbc1[:])
                aoTb = wpool.tile([128, 512], BF16, tag="aoTb", name="aoTb")
                for cc in range(4):
                    nc.gpsimd.dma_start(
                        aoTb[64:128, ts(cc, 128)], aoT1[:, ts(cc, 128)]
                    )
                rbc0 = wpool.tile([64, 512], FP32, tag="rbc0", name="rbc0")
                nc.vector.reciprocal_approx_fast(rbc0[:], psb0[:])
                nc.vector.tensor_mul(aoTb[0:64, :], aoU0[0:64, :], rbc0[:])
                return aoTb

            def emit_outproj(J, aoTb, last=False):
                for qq in range(4):
                    if last and qq % 2 == 1:
                        pso = ps_pj.tile([128, 512], FP32, tag="pj", name="pso")
                    else:
                        pso = ps_ep.tile([128, 512], FP32, tag="ep", name="pso")
                    nc.tensor.matmul(
                        pso[:],
                        lhsT=aoTb[:, ts(qq, 128)],
                        rhs=woT_s[:],
                        start=True,
                        stop=True,
                    )
                    osb = wpool.tile([128, 512], FP16, tag="os", bufs=3, name="osb")
                    dst = outp[ds(J * 512 + qq * 128, 128), :]
                    if qq % 2 == 0:
                        nc.scalar.copy(osb[:], pso[:])
                        if last and qq == 2:
                            nc.sync.dma_start(dst, osb[:])
                        else:
                            nc.scalar.dma_start(dst, osb[:])
                    else:
                        nc.vector.tensor_copy(osb[:], pso[:])
                        if last and qq == 3:
                            nc.sync.dma_start(dst, osb[:])
                        else:
                            nc.gpsimd.dma_start(dst, osb[:])

            # ---- fused main loop ----
            # DVE handles all diagonal chunks (mask folded into bias tile)
            # plus a share of off-diagonal chunks for load balance.
            pend = None  # (J, aoU0, aoU1) awaiting bcast+outproj
            pend2 = None  # (J, aoTb) awaiting outproj
            xt0 = emit_proj_dma(0)
            nc.gpsimd.dma_start(
                wv_s[:].rearrange("p (d c) -> p d c", d=4),
                wv.rearrange("(d p) c -> p d c", p=128),
            )
            xt_next = emit_proj_dma(1)
            nc.scalar.dma_start(woT_s[:], woT[:])
            # gpsimd const construction AFTER the prefetch DMAs are queued
            # (these ops would otherwise delay the x-tile halves on the
            # gpsimd DMA queue by several us at startup)
            for h in range(2):
                nc.gpsimd.memset(bias_t[:, ds(h * 512, 128)], MASKVAL)
                nc.gpsimd.affine_select(
                    out=bias_t[:, ds(h * 512, 128)],
                    in_=bias_t[:, ds(h * 512, 128)],
                    compare_op=ALU.is_gt,
                    fill=SBIAS,
                    base=0,
                    pattern=[[-1, 128]],
                    channel_multiplier=1,
                )
            nc.gpsimd.memset(mask_s[:], 0.0)
            nc.gpsimd.affine_select(
                out=mask_s[:],
                in_=mask_s[:],
                compare_op=ALU.is_gt,
                fill=1.0,
                base=0,
                pattern=[[-1, 128]],
                channel_multiplier=1,
            )
            for piece in range(4):
                emit_proj_piece(0, piece, xt0, pool=ps_sc, tag="sc")
            for J in range(nb):
                nkq = 4 * J + 4
                po0 = ps_pv.tile([65, 512], FP32, tag="pv0", name="po0")
                po1 = ps_pv.tile([65, 512], FP32, tag="pv1", name="po1")
                pts = {}
                last_scores = None
                for K in range(nkq + 3):
                    if K < nkq:
                        off = K * 128 - J * 512
                        n0 = max(off, 0)
                        w = 512 - n0
                        pssc = ps_sc.tile([128, 1024], FP32, tag="sc", name="pssc")
                        nc.tensor.matmul(
                            pssc[:, n0:512],
                            lhsT=kT_s[0:64, ts(K, 128)],
                            rhs=qT_s[0:64, ds(J * 512 + n0, w)],
                            start=True,
                            stop=True,
                        )
                        last_scores = nc.tensor.matmul(
                            pssc[:, 512 + n0 : 1024],
                            lhsT=kT_s[64:128, ts(K, 128)],
                            rhs=qT_s[64:128, ds(J * 512 + n0, w)],
                            start=True,
                            stop=True,
                        )
                        pt = wpool.tile([128, 1024], BF16, tag="pt", bufs=5, name="pt")
                        use_dve = (K % 2 == 1) if off < 0 else (n0 in (128, 384))
                        if use_dve:
                            if off >= 0:
                                src = pssc[:].rearrange("p (h n) -> p h n", h=2)[
                                    :, :, n0:512
                                ]
                                dst = pt[:].bitcast(U16).rearrange(
                                    "p (h n) -> p h n", h=2
                                )[:, :, n0:512]
                                bsl = bias_t[:].rearrange("p (h n) -> p h n", h=2)[
                                    :, :, 0:w
                                ]
                            else:
                                src, dst, bsl = pssc[:], pt[:].bitcast(U16), bias_p[:]
                            nc.vector.scalar_tensor_tensor(
                                dst, src, SMUL, bsl, ALU.mult, ALU.add
                            )
                        else:
                            src = pssc[:].rearrange("p (h n) -> p h n", h=2)[
                                :, :, n0:512
                            ]
                            dst = pt[:].rearrange("p (h n) -> p h n", h=2)[
                                :, :, n0:512
                            ]
                            nc.scalar.activation(
                                dst, src, AF.Exp, scale=inv_sqrt_d, bias=negoff[:]
                            )
                            if off >= 0:
                                nc.gpsimd.tensor_mul(
                                    pt[:, ds(n0, 128)], pt[:, ds(n0, 128)], mask_s[:]
                                )
                                nc.gpsimd.tensor_mul(
                                    pt[:, ds(512 + n0, 128)],
                                    pt[:, ds(512 + n0, 128)],
                                    mask_s[:],
                                )
                        pts[K] = (pt, n0, w)
                    if K == 0 and J + 2 < nb:
                        xt_nn = emit_proj_dma(J + 2)
                    if K == 5 and J + 1 < nb:
                        xt_next = xt_nn
                    if 1 <= K <= 4 and J + 1 < nb:
                        emit_proj_piece(J + 1, K - 1, xt_next)
                    if K == 3 and pend is not None:
                        pj, a0, a1 = pend
                        pend2 = (pj, emit_bcast(pj, a0, a1))
                        pend = None
                    if K == 5 and pend2 is not None:
                        pj, aoTb = pend2
                        emit_outproj(pj, aoTb)
                        pend2 = None
                    if K >= 3:
                        Kp = K - 3
                        pt_p, n0_p, w_p = pts.pop(Kp)
                        st = Kp == 0
                        sp = Kp == nkq - 1
                        pv0_mm = nc.tensor.matmul(
                            po0[0:65, ds(n0_p, w_p)],
                            lhsT=v_s[:, Kp, 0, :],
                            rhs=pt_p[:, ds(n0_p, w_p)],
                            start=st,
                            stop=sp,
                            skip_group_check=True,
                        )
                        if K < nkq and last_scores is not None:
                            add_dep_helper(
                                pv0_mm.ins,
                                last_scores.ins,
                                sync=False,
                                reason="pipeline skew",
                            )
                        nc.tensor.matmul(
                            po1[0:65, ds(n0_p, w_p)],
                            lhsT=v_s[:, Kp, 1, :],
                            rhs=pt_p[:, ds(512 + n0_p, w_p)],
                            start=st,
                            stop=sp,
                            skip_group_check=True,
                        )
                # free PV banks quickly, then queue the rest of the epilogue
                # into the next block's chunk loop
                if pend2 is not None:  # small blocks (J=0) may not reach K==4
                    pj, aoTb = pend2
                    emit_outproj(pj, aoTb)
                    pend2 = None
                pend = (J, *emit_aou(J, po0, po1))
            # drain the last block's epilogue
            pj, a0, a1 = pend
            aoTb = emit_bcast(pj, a0, a1, last=True)
            emit_outproj(pj, aoTb, last=True)
    nc.compile()
    return nc


def make_in_maps(x, W_QKV, W_O, t=T, n_cores=8):
    x = np.ascontiguousarray(np.asarray(x, dtype=np.float32))
    W_QKV = np.asarray(W_QKV, dtype=np.float32)
    W_O = np.asarray(W_O, dtype=np.float32)
    B = x.shape[0]
    bf16 = ml_dtypes.bfloat16
    xTs = [np.ascontiguousarray(x[b, :t].T).astype(bf16) for b in range(B)]
    in_maps = []
    for c in range(n_cores):
        b = c // 4
        g = c % 4
        hs = slice(2 * g * 64, 2 * g * 64 + 128)
        in_maps.append(
            {
                "xT": xTs[b],
                "wq": np.ascontiguousarray(W_QKV[0:512][hs].T).astype(bf16),
                "wk": np.ascontiguousarray(W_QKV[512:1024][hs].T).astype(bf16),
                "wv": np.ascontiguousarray(W_QKV[1024:1536][hs].T).astype(bf16),
                "woT": np.ascontiguousarray(W_O[:, hs].T).astype(bf16),
            }
        )
    return in_maps


def kernel(x, W_QKV, W_O):
    global LAST_RESULTS
    x = np.asarray(x, dtype=np.float32)
    B, t, _ = x.shape
    nc = build_program(t)
    in_maps = make_in_maps(x, W_QKV, W_O, t=t)
    res = run_bass_kernel_spmd(
        nc, in_maps, core_ids=list(range(8)), trace=TRACE
    )
    LAST_RESULTS = res
    parts = [r["outp"] for r in res.results]
    out = np.empty((B, t, DM), dtype=np.float32)
    for b in range(B):
        acc = np.zeros((t, DM), dtype=np.float64)
        for g in range(4):
            acc += parts[b * 4 + g]
        out[b] = acc.astype(np.float32)
    return out


# revision 33
# speedup vs baseline: 1.0130x; 1.0130x over previous
"""Causal self-attention TRN2 Bass kernel (v2).

Problem: B=2, T=4096, D_MODEL=512, N_HEADS=8, HEAD_DIM=64 (fp32).

Sharding (tensor+data parallel): 8 cores = 2 batches x 4 head-pairs.
Core c handles batch b = c//4 and heads (2g, 2g+1) with g = c%4, over the
full sequence. Each core computes a full-shape [T, 512] partial output
(its two heads' contribution through W_O); the host sums 4 partials per
batch.

v2 structure (single fused loop over query-blocks J of 512 queries; the
whole program is one software pipeline so the PE never idles long enough
for the HAM clock-gate to re-throttle it to 1.2 GHz):
  - the QKV projection for block J+1 is emitted piecewise INSIDE block
    J's chunk loop (iterations K=1..4) on a dedicated 1-bank PSUM tag,
    with the x-tile DMA split across the sync and gpsimd queues so input
    prefetch never queues behind epilogue DMAs.
  - attention: per 128-key chunk, row-tiled score pair (head0 rows 0:64,
    head1 rows 64:128, concurrent), then exp, alternating per chunk
    between ScalarE (exact: exp(s/8 - 4), with a multiplicative GpSimd
    causal-mask on diagonal blocks) and DVE (Schraudolph bit-trick: one
    scalar_tensor_tensor computes uint16(round(s*23.083 + 15511.7)),
    reinterpreted as bf16 ~= exp(s/8-4) within ~3%; for diagonal chunks
    the bias operand is a triangle tile whose masked entries hold 4000,
    making exp ~2^-96, i.e. exact-zero contribution). The global -4
    offset cancels in softmax normalization and keeps exp(max score)
    finite in bf16. Splitting exp across two engines removes the
    ~150us ScalarE wall a single-engine softmax would hit.
  - PV matmuls M=65 (64 dims + ones column accumulating denominators
    for free), software-pipelined 3 chunks behind the score matmuls.
  - epilogue per block pipelined into the NEXT block's chunk loop:
    aoU copies right after the last PV (frees single-buffered PSUM
    accumulators), denominator-row broadcast via K=1 matmuls, fp32
    reciprocals, bf16 normalize muls, head1 partition-shift via
    gpsimd-issued SBUF DMA, W_O projection, output staged through SBUF
    alternating ScalarE/DVE with engine-issued output DMAs.
"""

import math

import ml_dtypes
import numpy as np

import concourse.bass as bass
import concourse.mybir as mybir
import concourse.tile as tile
from concourse.tile import add_dep_helper
from concourse import bacc
from concourse.bass import ds, ts
from concourse.bass_utils import run_bass_kernel_spmd

FP32 = mybir.dt.float32
BF16 = mybir.dt.bfloat16
U16 = mybir.dt.uint16
FP16 = mybir.dt.float16
AF = mybir.ActivationFunctionType
ALU = mybir.AluOpType

T = 4096
DM = 512
QC = 512  # query-block width
KC = 128  # key-chunk width (partition dim)

# Schraudolph bf16-bit exp: bits = s_raw * SMUL + SBIAS, as uint16 -> bf16.
# value ~= exp(s_raw/8 - 4) (matches the ScalarE path's scale/bias).
LOG2E = 1.4426950408889634
SMUL = 128.0 * LOG2E / 8.0  # 23.0831...
SBIAS = 16256.0 - 5.61 - 4.0 * 128.0 * LOG2E  # 15511.73
MASKVAL = 4000.0  # masked-entry bias: exp -> ~2^-96, exactly negligible

TRACE = False
LAST_RESULTS = None


def build_program(t=T):
    assert t % QC == 0
    nb = t // QC
    nkc = t // KC
    nc = bacc.Bacc("TRN2", target_bir_lowering=False, debug=False)

    xT = nc.dram_tensor("xT", [DM, t], BF16, kind="ExternalInput").ap()
    wq = nc.dram_tensor("wq", [128, 512], BF16, kind="ExternalInput").ap()
    wk = nc.dram_tensor("wk", [128, 512], BF16, kind="ExternalInput").ap()
    wv = nc.dram_tensor("wv", [128, 512], BF16, kind="ExternalInput").ap()
    woT = nc.dram_tensor("woT", [128, DM], BF16, kind="ExternalInput").ap()
    outp = nc.dram_tensor("outp", [t, DM], FP16, kind="ExternalOutput").ap()

    with tile.TileContext(nc) as tc:
        with (
            tc.tile_pool(name="consts", bufs=1) as cpool,
            tc.tile_pool(name="persist", bufs=1) as ppool,
            tc.tile_pool(name="xtl", bufs=3) as xpool,
            tc.tile_pool(name="work", bufs=2) as wpool,
            tc.tile_pool(name="ps_sc", bufs=2, space="PSUM") as ps_sc,
            tc.tile_pool(name="ps_pv", bufs=1, space="PSUM") as ps_pv,
            tc.tile_pool(name="ps_ep", bufs=1, space="PSUM") as ps_ep,
            tc.tile_pool(name="ps_pj", bufs=1, space="PSUM") as ps_pj,
        ):
            # ---- constants ----
            wq_s = cpool.tile([128, 512], BF16, name="wq_s")
            wk_s = cpool.tile([128, 512], BF16, name="wk_s")
            wv_s = cpool.tile([128, 512], BF16, name="wv_s")
            woT_s = cpool.tile([128, 512], BF16, name="woT_s")
            nc.sync.dma_start(wq_s[:], wq[:])
            nc.gpsimd.dma_start(wk_s[:], wk[:])

            # Schraudolph bias tiles: plain (off-diagonal) via memset,
            # diagonal-masked variants host-precomputed and DMA'd
            bias_p = cpool.tile([128, 1024], FP32, name="bias_p")
            nc.vector.memset(bias_p[:], SBIAS)
            # diagonal bias tile, slice-relative: triangle (MASKVAL where
            # k > c) in cols [0,128), SBIAS elsewhere; same for both heads
            bias_t = cpool.tile([128, 1024], FP32, name="bias_t")
            nc.vector.memset(bias_t[:], SBIAS)

            # multiplicative causal mask for diagonal blocks of P^T [k, q]:
            # 1 where k <= q, 0 elsewhere (applied to exp output on GpSimd)
            mask_s = cpool.tile([128, 128], BF16, name="mask_s")

            # ones row at partition 64 for the K=1 denominator broadcasts
            ones_bc = cpool.tile([65, 64], BF16, name="ones_bc")
            nc.vector.memset(ones_bc[:], 1.0)

            # per-partition bias vector for the ScalarE exp path (-4 offset)
            negoff = cpool.tile([128, 1], FP32, name="negoff")
            nc.vector.memset(negoff[:], -4.0)

            # ---- persistent activations ----
            qT_s = ppool.tile([128, t], BF16, name="qT_s")
            kT_s = ppool.tile([128, t], BF16, name="kT_s")
            # V natural per chunk/head: [key-in-chunk, chunk, head, 65]
            # (64 dims + ones column; memset 1.0 once)
            v_s = ppool.tile([128, nkc, 2, 65], BF16, name="v_s")
            nc.vector.memset(v_s[:], 1.0)

            inv_sqrt_d = 1.0 / math.sqrt(64.0)

            def emit_proj_dma(J):
                xt = xpool.tile([128, 4, 512], BF16, tag="xt", name="xt")
                src = xT.rearrange("(d p) t -> p d t", p=128)[:, :, ts(J, 512)]
                nc.sync.dma_start(xt[:, 0:2, :], src[:, 0:2, :])
                nc.gpsimd.dma_start(xt[:, 2:4, :], src[:, 2:4, :])
                return xt

            def emit_proj_piece(J, piece, xt, pool=None, tag="pj"):
                # piece 0: Q-proj, 1: K-proj, 2: V tokens 0:256, 3: V 256:512
                pool = pool or ps_pj
                if piece in (0, 1):
                    w_s = (wq_s, wk_s)[piece]
                    psq = pool.tile([128, 512], FP32, tag=tag, name="psq")
                    for d in range(4):
                        nc.tensor.matmul(
                            psq[:],
                            lhsT=w_s[:, ts(d, 128)],
                            rhs=xt[:, d, :],
                            start=(d == 0),
                            stop=(d == 3),
                        )
                    dst = (qT_s, kT_s)[piece]
                    nc.scalar.copy(dst[:, ts(J, 512)], psq[:])
                else:
                    psv = pool.tile([128, 512], FP32, tag=tag, name="psv")
                    for tt2 in range(2):
                        tt = (piece - 2) * 2 + tt2
                        for d in range(4):
                            nc.tensor.matmul(
                                psv[:, ds(tt2 * 256, 128)],
                                lhsT=xt[:, d, ts(tt, 128)],
                                rhs=wv_s[:, ts(d, 128)],
                                start=(d == 0),
                                stop=(d == 3),
                            )
                    for tt2 in range(2):
                        kk = 4 * J + (piece - 2) * 2 + tt2
                        nc.vector.tensor_copy(
                            v_s[:, kk, :, 0:64],
                            psv[:, ds(tt2 * 256, 128)].rearrange(
                                "p (h d) -> p h d", h=2
                            ),
                        )

            # epilogue state: (J, po0, po1) -> after aoU copies: (J, aoU0, aoU1)
            def emit_aou(J, po0, po1):
                aoU0 = wpool.tile([65, 512], BF16, tag="ao0", name="aoU0")
                aoU1 = wpool.tile([65, 512], BF16, tag="ao1", name="aoU1")
                nc.scalar.copy(aoU0[:], po0[:])
                nc.vector.tensor_copy(aoU1[:], po1[:])
                return aoU0, aoU1

            def emit_bcast(J, aoU0, aoU1, last=False):
                # broadcast each denominator row to 64 partitions (K=1
                # matmuls), reciprocal, then normalize (all base-partition 0)
                psb0 = ps_ep.tile([64, 512], FP32, tag="ep", name="psb0")
                nc.tensor.matmul(
                    psb0[:],
                    lhsT=ones_bc[64:65, :],
                    rhs=aoU0[64:65, :],
                    start=True,
                    stop=True,
                )
                if last:
                    psb1 = ps_pj.tile([64, 512], FP32, tag="pj", name="psb1")
                else:
                    psb1 = ps_ep.tile([64, 512], FP32, tag="ep", name="psb1")
                nc.tensor.matmul(
                    psb1[:],
                    lhsT=ones_bc[64:65, :],
                    rhs=aoU1[64:65, :],
                    start=True,
                    stop=True,
                )
                rbc1 = wpool.tile([64, 512], FP32, tag="rbc1", name="rbc1")
                nc.vector.reciprocal_approx_fast(rbc1[:], psb1[:])
                aoT1 = wpool.tile([64, 512], BF16, tag="aoT1", name="aoT1")
                nc.vector.tensor_mul(aoT1[:], aoU1[0:64, :], rbc1[:])
                aoTb = wpool.tile([128, 512], BF16, tag="aoTb", name="aoTb")
                nc.gpsimd.dma_start(aoTb[64:128, :], aoT1[:])
                rbc0 = wpool.tile([64, 512], FP32, tag="rbc0", name="rbc0")
                nc.vector.reciprocal_approx_fast(rbc0[:], psb0[:])
                nc.vector.tensor_mul(aoTb[0:64, :], aoU0[0:64, :], rbc0[:])
                return aoTb

            def emit_outproj(J, aoTb, last=False):
                for qq in range(4):
                    if last and qq % 2 == 1:
                        pso = ps_pj.tile([128, 512], FP32, tag="pj", name="pso")
                    else:
                        pso = ps_ep.tile([128, 512], FP32, tag="ep", name="pso")
                    nc.tensor.matmul(
                        pso[:],
                        lhsT=aoTb[:, ts(qq, 128)],
                        rhs=woT_s[:],
                        start=True,
                        stop=True,
                    )
                    osb = wpool.tile([128, 512], FP16, tag="os", bufs=3, name="osb")
                    if qq % 2 == 0:
                        nc.scalar.copy(osb[:], pso[:])
                        nc.scalar.dma_start(outp[ds(J * 512 + qq * 128, 128), :], osb[:])
                    else:
                        nc.vector.tensor_copy(osb[:], pso[:])
                        nc.gpsimd.dma_start(outp[ds(J * 512 + qq * 128, 128), :], osb[:])

            # ---- fused main loop ----
            # DVE handles all diagonal chunks (mask folded into bias tile)
            # plus a share of off-diagonal chunks for load balance.
            pend = None  # (J, aoU0, aoU1) awaiting bcast+outproj
            pend2 = None  # (J, aoTb) awaiting outproj
            xt0 = emit_proj_dma(0)
            nc.gpsimd.dma_start(wv_s[:], wv[:])
            xt_next = emit_proj_dma(1)
            nc.scalar.dma_start(woT_s[:], woT[:])
            # gpsimd const construction AFTER the prefetch DMAs are queued
            # (these ops would otherwise delay the x-tile halves on the
            # gpsimd DMA queue by several us at startup)
            for h in range(2):
                nc.gpsimd.memset(bias_t[:, ds(h * 512, 128)], MASKVAL)
                nc.gpsimd.affine_select(
                    out=bias_t[:, ds(h * 512, 128)],
                    in_=bias_t[:, ds(h * 512, 128)],
                    compare_op=ALU.is_gt,
                    fill=SBIAS,
                    base=0,
                    pattern=[[-1, 128]],
                    channel_multiplier=1,
                )
            nc.gpsimd.memset(mask_s[:], 0.0)
            nc.gpsimd.affine_select(
                out=mask_s[:],
                in_=mask_s[:],
                compare_op=ALU.is_gt,
                fill=1.0,
                base=0,
                pattern=[[-1, 128]],
                channel_multiplier=1,
            )
            for piece in range(4):
                emit_proj_piece(0, piece, xt0, pool=ps_sc, tag="sc")
            for J in range(nb):
                nkq = 4 * J + 4
                po0 = ps_pv.tile([65, 512], FP32, tag="pv0", name="po0")
                po1 = ps_pv.tile([65, 512], FP32, tag="pv1", name="po1")
                pts = {}
                last_scores = None
                for K in range(nkq + 3):
                    if K < nkq:
                        off = K * 128 - J * 512
                        n0 = max(off, 0)
                        w = 512 - n0
                        pssc = ps_sc.tile([128, 1024], FP32, tag="sc", name="pssc")
                        nc.tensor.matmul(
                            pssc[:, n0:512],
                            lhsT=kT_s[0:64, ts(K, 128)],
                            rhs=qT_s[0:64, ds(J * 512 + n0, w)],
                            start=True,
                            stop=True,
                        )
                        last_scores = nc.tensor.matmul(
                            pssc[:, 512 + n0 : 1024],
                            lhsT=kT_s[64:128, ts(K, 128)],
                            rhs=qT_s[64:128, ds(J * 512 + n0, w)],
                            start=True,
                            stop=True,
                        )
                        pt = wpool.tile([128, 1024], BF16, tag="pt", bufs=5, name="pt")
                        use_dve = (K % 2 == 1) if off < 0 else (n0 in (128, 384))
                        if use_dve:
                            if off >= 0:
                                src = pssc[:].rearrange("p (h n) -> p h n", h=2)[
                                    :, :, n0:512
                                ]
                                dst = pt[:].bitcast(U16).rearrange(
                                    "p (h n) -> p h n", h=2
                                )[:, :, n0:512]
                                bsl = bias_t[:].rearrange("p (h n) -> p h n", h=2)[
                                    :, :, 0:w
                                ]
                            else:
                                src, dst, bsl = pssc[:], pt[:].bitcast(U16), bias_p[:]
                            nc.vector.scalar_tensor_tensor(
                                dst, src, SMUL, bsl, ALU.mult, ALU.add
                            )
                        else:
                            src = pssc[:].rearrange("p (h n) -> p h n", h=2)[
                                :, :, n0:512
                            ]
                            dst = pt[:].rearrange("p (h n) -> p h n", h=2)[
                                :, :, n0:512
                            ]
                            nc.scalar.activation(
                                dst, src, AF.Exp, scale=inv_sqrt_d, bias=negoff[:]
                            )
                            if off >= 0:
                                nc.gpsimd.tensor_mul(
                                    pt[:, ds(n0, 128)], pt[:, ds(n0, 128)], mask_s[:]
                                )
                                nc.gpsimd.tensor_mul(
                                    pt[:, ds(512 + n0, 128)],
                                    pt[:, ds(512 + n0, 128)],
                                    mask_s[:],
                                )
                        pts[K] = (pt, n0, w)
                    if K == 0 and J + 2 < nb:
                        xt_nn = emit_proj_dma(J + 2)
                    if K == 5 and J + 1 < nb:
                        xt_next = xt_nn
                    if 1 <= K <= 4 and J + 1 < nb:
                        emit_proj_piece(J + 1, K - 1, xt_next)
                    if K == 3 and pend is not None:
                        pj, a0, a1 = pend
                        pend2 = (pj, emit_bcast(pj, a0, a1))
                        pend = None
                    if K == 5 and pend2 is not None:
                        pj, aoTb = pend2
                        emit_outproj(pj, aoTb)
                        pend2 = None
                    if K >= 3:
                        Kp = K - 3
                        pt_p, n0_p, w_p = pts.pop(Kp)
                        st = Kp == 0
                        sp = Kp == nkq - 1
                        pv0_mm = nc.tensor.matmul(
                            po0[0:65, ds(n0_p, w_p)],
                            lhsT=v_s[:, Kp, 0, :],
                            rhs=pt_p[:, ds(n0_p, w_p)],
                            start=st,
                            stop=sp,
                            skip_group_check=True,
                        )
                        if K < nkq and last_scores is not None:
                            add_dep_helper(
                                pv0_mm.ins,
                                last_scores.ins,
                                sync=False,
                                reason="pipeline skew",
                            )
                        nc.tensor.matmul(
                            po1[0:65, ds(n0_p, w_p)],
                            lhsT=v_s[:, Kp, 1, :],
                            rhs=pt_p[:, ds(512 + n0_p, w_p)],
                            start=st,
                            stop=sp,
                            skip_group_check=True,
                        )
                # free PV banks quickly, then queue the rest of the epilogue
                # into the next block's chunk loop
                if pend2 is not None:  # small blocks (J=0) may not reach K==4
                    pj, aoTb = pend2
                    emit_outproj(pj, aoTb)
                    pend2 = None
                pend = (J, *emit_aou(J, po0, po1))
            # drain the last block's epilogue
            pj, a0, a1 = pend
            aoTb = emit_bcast(pj, a0, a1, last=True)
            emit_outproj(pj, aoTb, last=True)
    nc.compile()
    return nc


def make_in_maps(x, W_QKV, W_O, t=T, n_cores=8):
    x = np.ascontiguousarray(np.asarray(x, dtype=np.float32))
    W_QKV = np.asarray(W_QKV, dtype=np.float32)
    W_O = np.asarray(W_O, dtype=np.float32)
    B = x.shape[0]
    bf16 = ml_dtypes.bfloat16
    xTs = [np.ascontiguousarray(x[b, :t].T).astype(bf16) for b in range(B)]
    in_maps = []
    for c in range(n_cores):
        b = c // 4
        g = c % 4
        hs = slice(2 * g * 64, 2 * g * 64 + 128)

        def wprep(w):
            wT = np.asarray(w[hs].T, dtype=np.float32)  # [512 dm, 128]
            return np.ascontiguousarray(
                wT.reshape(4, 128, 128).transpose(1, 0, 2).reshape(128, 512)
            ).astype(bf16)

        in_maps.append(
            {
                "xT": xTs[b],
                "wq": wprep(W_QKV[0:512]),
                "wk": wprep(W_QKV[512:1024]),
                "wv": wprep(W_QKV[1024:1536]),
                "woT": np.ascontiguousarray(W_O[:, hs].T).astype(bf16),
            }
        )
    return in_maps


def kernel(x, W_QKV, W_O):
    global LAST_RESULTS
    x = np.asarray(x, dtype=np.float32)
    B, t, _ = x.shape
    nc = build_program(t)
    in_maps = make_in_maps(x, W_QKV, W_O, t=t)
    res = run_bass_kernel_spmd(
        nc, in_maps, core_ids=list(range(8)), trace=TRACE
    )
    LAST_RESULTS = res
    parts = [r["outp"] for r in res.results]
    out = np.empty((B, t, DM), dtype=np.float32)
    for b in range(B):
        acc = np.zeros((t, DM), dtype=np.float64)
        for g in range(4):
            acc += parts[b * 4 + g]
        out[b] = acc.astype(np.float32)
    return out
